# revision 1
# baseline (speedup 1.0000x reference)
"""ANFIS forward kernel for Trainium2, 8-core data-parallel. v6.

Algebra per row n (see reference):
    l_r = sum_d [2*c*a*x - a*x^2] - k_r;  s_r = exp(l_r)
    G_f = sum_r s_r * Chat[r,f]   (f = (i,o) products + S feature)
    U_o = sum_i xhat_i * G_(i,o);  out = softmax_o(U / (S + eps))

Layout: row n of a core's 16384-row slice -> (p, t) = (n // 128, n % 128).
Host supplies two tensors per core, each loaded in 4 contiguous
quarter-DMAs (one descriptor per partition per quarter):
  xt: [x, x^2] rows (MC, 32)  -- feeds PE transposes; one (128,128)
      transpose covers a whole group (4 tiles x 32 cols).
  xr: [x, 1]   rows (MC, 17)  -- feeds the DVE broadcast multiply.
Quad frontend: 4 groups share one PSUM transpose tile, one ACT
PSUM->SBUF copy, one ACT exp. Per group: M1 (fp32) -> logits,
M2 (f32r, stationary strengths) -> G in PSUM, DVE multiply by xhat,
DVE strided reduce -> U, ACT extracts S+eps. Softmax batched over 32
tiles: reciprocals on DVE, broadcast multiplies on Pool (GpSimd), exp
on ACT. Output stores per meta (4 contiguous DMAs).
"""

import numpy as np

N, D, R, O = 131072, 16, 32, 10
EPS = 1e-8
NCORES = 8
MC = N // NCORES          # rows per core = 16384
TPG = 4                   # tiles (of 128 rows) per group
GROUP = 128 * TPG         # 512 rows per group
NG = MC // GROUP          # 32 groups per core
QUAD = 2                  # groups per frontend batch
NQUAD = NG // QUAD        # 16
META = 8                  # groups per softmax batch
NMETA = NG // META        # 4

DI = D + 1                # 17: x dims + ones
DT = 2 * D                # 32: [x, x^2] row width
F = DI * O                # 170 product features
FS = F + 1                # 171: + strength-sum feature
FPAD = 256                # per-tile feature stride in G4 (bank alignment)
NT = MC // 128            # 128 tiles per core
# input chunks (tile_start, n_tiles), packed [xt | xr] per partition per
# chunk; small first chunk -> early compute start, still one DMA each.
CH = [(0, 24), (24, 104)]
RW = DT + DI              # 49 cols per tile in the packed tensor
CB = [s * RW for s, n in CH]  # chunk base columns


def _build_constants(centers, sigmas, coeffs):
    a = 1.0 / (2.0 * sigmas.astype(np.float64) ** 2)          # (R,D)
    c = centers.astype(np.float64)

    # WL4: lhsT for M1. out partition (j,r) = j*32+r.
    # rhs partition ordinal (from the per-group transpose of [x, x^2]
    # rows) is (j, s, d) = j*32 + s*16 + d.
    wl4 = np.zeros((128, 128), np.float64)
    for j in range(TPG):
        for r in range(R):
            pi = j * R + r
            for d in range(D):
                wl4[j * 32 + 0 * 16 + d, pi] = 2.0 * c[r, d] * a[r, d]   # x
                wl4[j * 32 + 1 * 16 + d, pi] = -a[r, d]                  # x^2
    negk = -(c * c * a).sum(axis=1)                            # (R,)
    negk4 = np.tile(negk, TPG).reshape(128, 1)

    # Chat (R, 171): features f = o*17+i (i=16 -> bias row), f=170 -> ones.
    chat = np.zeros((R, FS), np.float64)
    chat[:, :F] = coeffs.astype(np.float64).transpose(0, 2, 1).reshape(
        R, FS - 1)                                              # (R,10*17)
    chat[:, F] = 1.0
    # C2D4 (128, 1024): [(j,r), j'*256+f] = delta_jj' * chat[r,f]
    c2d4 = np.zeros((128, TPG * FPAD), np.float64)
    for j in range(TPG):
        c2d4[j * R:(j + 1) * R, j * FPAD:j * FPAD + FS] = chat
    return (wl4.astype(np.float32), negk4.astype(np.float32),
            c2d4.astype(np.float32))


def _build_bass():
    import concourse.bacc as bacc
    import concourse.mybir as mybir
    from concourse import masks
    from concourse.tile import TileContext

    f32 = mybir.dt.float32
    f32r = mybir.dt.float32r
    AX = mybir.AxisListType
    ALU = mybir.AluOpType
    ACTF = mybir.ActivationFunctionType

    nc = bacc.Bacc("TRN2", target_bir_lowering=False, debug=False)
    xall_d = nc.declare_dram_parameter("xall", [128, NT * RW], f32,
                                       isOutput=False)
    cst_d = nc.declare_dram_parameter("cst", [128, 129], f32, isOutput=False)
    c2d4_d = nc.declare_dram_parameter("c2d4", [128, TPG * FPAD], f32r,
                                       isOutput=False)
    yout = nc.declare_dram_parameter("yout", [MC, O], f32, isOutput=True)

    youtv = yout[:, :].rearrange("(p t) o -> p t o", p=128)

    with TileContext(nc) as tc:
        with (
            tc.tile_pool(name="const", bufs=1) as cpool,
            tc.tile_pool(name="front", bufs=2) as fpool,
            tc.tile_pool(name="work", bufs=3) as wpool,
            tc.tile_pool(name="stage", bufs=3) as spool,
            tc.tile_pool(name="ps_t", bufs=1, space="PSUM") as ps_t,
            tc.tile_pool(name="ps_l", bufs=1, space="PSUM") as ps_l,
            tc.tile_pool(name="ps_g", bufs=3, space="PSUM") as ps_g,
        ):
            ident = cpool.tile([128, 128], f32)
            masks.make_identity(nc, ident[:])

            # first transpose-source quarter goes out first so compute can
            # start as early as possible.
            xall = cpool.tile([128, NT * RW], f32)
            xt_q = [xall[:, CB[i]:CB[i] + CH[i][1] * DT]
                    for i in range(len(CH))]
            xr_q = [xall[:, CB[i] + CH[i][1] * DT:CB[i] + CH[i][1] * RW]
                    for i in range(len(CH))]
            o_all = cpool.tile([128, NT * O], f32)
            nc.sync.dma_start(out=xall[:, 0:CB[1]],
                              in_=xall_d[:, 0:CB[1]])
            cst = cpool.tile([128, 129], f32)
            nc.sync.dma_start(out=cst[:], in_=cst_d[:, :])
            negk4 = cst[:, 0:1]
            wl4 = cst[:, 1:129]

            c2d4 = cpool.tile([128, TPG * FPAD], f32r)
            nc.sync.dma_start(out=c2d4[:], in_=c2d4_d[:, :])
            nc.sync.dma_start(out=xall[:, CB[1]:NT * RW],
                              in_=xall_d[:, CB[1]:NT * RW])

            for m in range(NMETA):
                u32 = spool.tile([128, META * TPG * O], f32, tag="u32")
                s32 = spool.tile([128, META * TPG], f32, tag="s32")
                e32 = spool.tile([128, META * TPG * O], f32, tag="e32")
                se32 = spool.tile([128, META * TPG], f32, tag="se32")


                for qq in range(META // QUAD):
                    Q = m * (META // QUAD) + qq   # quad index
                    t0 = Q * QUAD * TPG           # first tile of this quad
                    h = next(i for i, (s, n) in enumerate(CH)
                             if s <= t0 < s + n)  # input chunk
                    tb = t0 - CH[h][0]            # tile base within chunk
                    # -- 4 per-group transposes into one PSUM tile ---------
                    xtp = ps_t.tile([128, QUAD * 128], f32, tag="xtp")
                    for k in range(QUAD):
                        nc.tensor.transpose(
                            xtp[:, 128 * k:128 * (k + 1)],
                            xt_q[h][:, DT * (tb + TPG * k):
                                    DT * (tb + TPG * (k + 1))],
                            ident[:])
                    # -- one PSUM->SBUF copy for the whole quad ------------
                    xs = fpool.tile([128, QUAD * 128], f32, tag="xs")
                    nc.scalar.activation(xs[:], xtp[:], ACTF.Copy)
                    # -- M1 x4 into one PSUM bank, one exp -----------------
                    l16 = ps_l.tile([128, QUAD * 128], f32, tag="l16")
                    for k in range(QUAD):
                        nc.tensor.matmul(
                            l16[:, 128 * k:128 * (k + 1)], lhsT=wl4,
                            rhs=xs[:, 128 * k:128 * (k + 1)],
                            start=True, stop=True)
                    sst = fpool.tile([128, QUAD * 128], f32r, tag="sst")
                    nc.scalar.activation(sst[:], l16[:], ACTF.Exp,
                                         bias=negk4, scale=1.0)

                    for k in range(QUAD):
                        q = qq * QUAD + k         # group within meta
                        # -- M2 -------------------------------------------
                        g4 = ps_g.tile([128, TPG * FPAD], f32, tag="g4")
                        nc.tensor.matmul(
                            g4[:, 0:512], lhsT=sst[:, 128 * k:128 * (k + 1)],
                            rhs=c2d4[:, 0:512], start=True, stop=True)
                        nc.tensor.matmul(
                            g4[:, 512:1024],
                            lhsT=sst[:, 128 * k:128 * (k + 1)],
                            rhs=c2d4[:, 512:1024], start=True, stop=True)
                        # -- P = G * xhat (bcast over o) ------------------
                        p4 = wpool.tile([128, TPG * F], f32, tag="p4")
                        p4v = p4[:].rearrange("p (j o i) -> p j o i",
                                              j=TPG, o=O)
                        g4v = g4[:].rearrange("p (j f) -> p j f",
                                              j=TPG)[:, :, 0:F].rearrange(
                            "p j (o i) -> p j o i", o=O)
                        xrv = xr_q[h].rearrange("p (t c) -> p t c", c=DI)
                        xhv = xrv[:, tb + TPG * k:tb + TPG * (k + 1),
                                  :].unsqueeze(2).broadcast_to(
                            [128, TPG, O, DI])
                        nc.vector.tensor_tensor(p4v, g4v, xhv, ALU.mult)
                        # -- S+eps extract (ACT), U = sum_i P (DVE) -------
                        nc.scalar.activation(
                            s32[:, q * TPG:(q + 1) * TPG],
                            g4[:].rearrange("p (j f) -> p j f",
                                            j=TPG)[:, :, F:F + 1].squeeze(2),
                            ACTF.Copy, bias=EPS)
                        nc.vector.tensor_reduce(
                            u32[:, q * TPG * O:(q + 1) * TPG * O].rearrange(
                                "p (j o) -> p j o", j=TPG),
                            p4v,
                            axis=AX.X, op=ALU.add)

                # -- batched normalize + softmax over 32 tiles -------------
                nc.vector.reciprocal(s32[:], s32[:])
                u32v = u32[:].rearrange("p (g o) -> p g o", o=O)
                s32b = s32[:].unsqueeze(2).broadcast_to(
                    [128, META * TPG, O])
                eng_tt = nc.vector if m == NMETA - 1 else nc.gpsimd
                eng_tt.tensor_tensor(u32v, u32v, s32b, ALU.mult)
                nc.scalar.activation(e32[:], u32[:], ACTF.Exp)
                nc.vector.tensor_reduce(
                    se32[:], e32[:].rearrange("p (g o) -> p g o", o=O),
                    axis=AX.X, op=ALU.add)
                nc.vector.reciprocal(se32[:], se32[:])
                se32b = se32[:].unsqueeze(2).broadcast_to(
                    [128, META * TPG, O])
                eng_tt.tensor_tensor(
                    o_all[:, m * META * TPG * O:(m + 1) * META * TPG * O
                          ].rearrange("p (g o) -> p g o", o=O),
                    e32[:].rearrange("p (g o) -> p g o", o=O),
                    se32b, ALU.mult)

            # -- one contiguous store for all 16384 rows -------------------
            nc.sync.dma_start(
                out=youtv[:, :, :],
                in_=o_all[:].rearrange("p (t o) -> p t o", o=O))
    nc.compile()
    return nc


def _pack(xt2c, xaugc):
    """(128, NT*RW): per partition, per chunk [xt-tiles | xr-tiles]."""
    xtr = xt2c.reshape(128, NT, DT)
    xrr = xaugc.reshape(128, NT, DI)
    parts = []
    for s, n in CH:
        parts.append(xtr[:, s:s + n].reshape(128, -1))
        parts.append(xrr[:, s:s + n].reshape(128, -1))
    return np.ascontiguousarray(np.concatenate(parts, axis=1))


_NC_CACHE = None


def kernel(X, centers, sigmas, coeffs):
    global _NC_CACHE
    from concourse import bass_utils

    X = np.asarray(X, np.float32)
    wl4, negk4, c2d4 = _build_constants(
        np.asarray(centers, np.float32),
        np.asarray(sigmas, np.float32),
        np.asarray(coeffs, np.float32))
    cst = np.concatenate([negk4, wl4], axis=1)

    xaug = np.ones((N, DI), np.float32)
    xaug[:, 0:D] = X
    xt2 = np.empty((N, DT), np.float32)
    xt2[:, 0:D] = X
    xt2[:, D:DT] = X * X

    if _NC_CACHE is None:
        _NC_CACHE = _build_bass()
    nc = _NC_CACHE

    in_maps = []
    for c in range(NCORES):
        in_maps.append({
            "xall": _pack(xt2[c * MC:(c + 1) * MC],
                          xaug[c * MC:(c + 1) * MC]),
            "cst": cst, "c2d4": c2d4,
        })
    res = bass_utils.run_bass_kernel_spmd(nc, in_maps, list(range(NCORES)))
    return np.concatenate([r["yout"] for r in res.results], axis=0)



# revision 28
# speedup vs baseline: 1.3682x; 1.3682x over previous
"""ANFIS forward kernel for Trainium2, 8-core data-parallel. v7.

Algebra per row n (see reference):
    l_r = sum_d [2*c*a*x - a*x^2] - k_r;  s_r = exp(l_r)
    G_f = sum_r s_r * Chat[r,f]   (f = (o,i) products, i innermost)
    U_o = sum_i xhat_i * G_(o,i);  out = softmax_o(U / (S + eps))

v7 changes vs v6:
  - Host supplies the PE-transpose layout directly (xst), killing the
    on-chip transposes and their PSUM->SBUF copies.
  - M1 is one f32r matmul per 4-group batch (256+ wide -> 1 cycle/row).
  - S+eps comes from tiny per-group PE matmuls accumulated into a
    per-meta PSUM tile (eps seeded by a ones x eps/128 matmul), so the
    32 per-group ACT extracts become one DVE reciprocal per meta.
  - G is copied PSUM->SBUF by the otherwise-idle ACT engine with a cast
    to bf16; the big per-row multiply then runs in the DVE 2x_1p mode,
    and the i-reduction is split between DVE tensor_reduce and Pool
    tensor_tensor fold chains (Pool cannot touch PSUM, but p4 is SBUF).
  - Per-meta output stores instead of one tail store.

Layout: row n of a core's 16384-row slice -> (p, t) = (n // 128, n % 128).
Group g = tiles [4g, 4g+4); meta m = groups [8m, 8m+8) = tiles [32m, 32m+32).
"""

import numpy as np

N, D, R, O = 131072, 16, 32, 10
EPS = 1e-8
NCORES = 8
MC = N // NCORES          # rows per core = 16384
TPG = 4                   # tiles (of 128 rows) per group
NG = 32                   # groups per core
MB = 4                    # groups per M1 batch
META = 8                  # groups per softmax batch
NMETA = NG // META        # 4

DI = D + 1                # 17: x dims + ones
F = O * DI                # 170 product features, f = o*17 + i
FPAD = 256                # per-j feature stride in g4 (bank alignment)
NT = MC // 128            # 128 tiles per core

# input chunks in groups: early small chunk -> early compute start
CH = [(0, 4), (4, 4), (8, 8), (16, 16)]

# softmax batches, in groups; smaller ones at the end shorten the drain
METAS = [8, 8, 8, 4, 4]

# group PAIRS whose i-reduction runs as a Pool fold chain (rest: DVE
# reduce). None in the last two metas: Pool drains slowly at the tail.
POOL_RED = frozenset({1, 3, 5, 7, 9, 11, 12, 14})
# metas whose se-reduce runs on Pool (fold) instead of DVE
POOL_SE = frozenset()


def _build_constants(centers, sigmas, coeffs):
    a = 1.0 / (2.0 * sigmas.astype(np.float64) ** 2)          # (R,D)
    c = centers.astype(np.float64)

    # WL4: lhsT for M1. out partition (j,r) = j*32+r; input partition
    # (j,s,d) = j*32 + s*16 + d  (s=0: x, s=1: x^2).
    wl4 = np.zeros((128, 128), np.float64)
    for j in range(TPG):
        for r in range(R):
            pi = j * R + r
            for d in range(D):
                wl4[j * 32 + 0 * 16 + d, pi] = 2.0 * c[r, d] * a[r, d]   # x
                wl4[j * 32 + 1 * 16 + d, pi] = -a[r, d]                  # x^2
    negk = -(c * c * a).sum(axis=1)                            # (R,)
    negk4 = np.tile(negk, TPG).reshape(128, 1)

    # Chat (R, 170): f = o*17+i (i=16 -> bias row)
    chat = coeffs.astype(np.float64).transpose(0, 2, 1).reshape(R, F)
    # C2D4 (128, 1024): [(j,r), j'*256+f] = delta_jj' * chat[r,f]
    c2d4 = np.zeros((128, TPG * FPAD), np.float64)
    for j in range(TPG):
        c2d4[j * R:(j + 1) * R, j * FPAD:j * FPAD + F] = chat
    # sS rhs (128, 4): [(j,r), j'] = delta_jj' -> S_j per group
    srhs = np.zeros((128, TPG), np.float64)
    for j in range(TPG):
        srhs[j * R:(j + 1) * R, j] = 1.0
    cst = np.concatenate([negk4, wl4, srhs], axis=1)           # (128, 133)
    return cst.astype(np.float32), c2d4.astype(np.float32)


def _build_bass():
    import concourse.bacc as bacc
    import concourse.mybir as mybir
    from concourse.tile import TileContext

    f32 = mybir.dt.float32
    f32r = mybir.dt.float32r
    bf16 = mybir.dt.bfloat16
    AX = mybir.AxisListType
    ALU = mybir.AluOpType
    ACTF = mybir.ActivationFunctionType

    nc = bacc.Bacc("TRN2", target_bir_lowering=False, debug=False)
    xst_d = nc.declare_dram_parameter("xst", [128, NG * 128], f32r,
                                      isOutput=False)
    fp16 = mybir.dt.float16
    xrb_d = nc.declare_dram_parameter("xrb", [128, NT * DI], fp16,
                                      isOutput=False)
    cst_d = nc.declare_dram_parameter("cst", [128, 133], f32r, isOutput=False)
    c2d4_d = nc.declare_dram_parameter("c2d4", [128, TPG * FPAD], f32r,
                                       isOutput=False)
    yout = nc.declare_dram_parameter("yout", [MC, O], f32, isOutput=True)

    youtv = yout[:, :].rearrange("(p t) o -> p t o", p=128)

    with TileContext(nc) as tc:
        with (
            tc.tile_pool(name="const", bufs=1) as cpool,
            tc.tile_pool(name="sst", bufs=2) as epool,
            tc.tile_pool(name="work", bufs=3) as wpool,
            tc.tile_pool(name="meta", bufs=2) as mpool,
            tc.tile_pool(name="ps_l", bufs=1, space="PSUM") as ps_l,
            tc.tile_pool(name="ps_g", bufs=3, space="PSUM") as ps_g,
            tc.tile_pool(name="ps_s", bufs=1, space="PSUM") as ps_s,
        ):
            # constants: cst via the Pool SWDGE path (its descriptor-gen
            # doesn't hold HWDGE), xst chunk0 first in the HWDGE queue.
            cst = cpool.tile([128, 133], f32r)
            nc.gpsimd.dma_start(out=cst[:], in_=cst_d[:, :])
            negk4 = cst[:, 0:1]
            wl4 = cst[:, 1:129]
            srhs = cst[:, 129:133]
            xst = cpool.tile([128, NG * 128], f32r)
            xrb = cpool.tile([128, NT * DI], fp16)
            c2d4 = cpool.tile([128, TPG * FPAD], f32r)
            ones128 = cpool.tile([128, 128], f32)
            epscol = cpool.tile([128, R], f32)
            nc.vector.memset(ones128[:], 1.0)
            nc.vector.memset(epscol[:], EPS / 128.0)
            # hoist the ACT exp-table load out of the critical path
            dummy = cpool.tile([128, 1], f32)
            nc.scalar.activation(dummy[:], epscol[:, 0:1], ACTF.Exp)
            for i, (s, n) in enumerate(CH):
                nc.sync.dma_start(out=xst[:, s * 128:(s + n) * 128],
                                  in_=xst_d[:, s * 128:(s + n) * 128])
                nc.sync.dma_start(out=xrb[:, s * TPG * DI:(s + n) * TPG * DI],
                                  in_=xrb_d[:, s * TPG * DI:(s + n) * TPG * DI])
                if i == 0:
                    nc.sync.dma_start(out=c2d4[:, 0:512],
                                      in_=c2d4_d[:, 0:512])
                    nc.sync.dma_start(out=c2d4[:, 512:1024],
                                      in_=c2d4_d[:, 512:1024])
            xrv = xrb[:].rearrange("p (t c) -> p t c", c=DI)

            # ---- phase 1: all M1 batches + exps -> sst_all in SBUF -------
            # Keeps the steady-state ACT stream homogeneous (copies only):
            # an exp interleaved between copies stalls the whole in-order
            # ACT queue on its M1 dependency.
            sst_all = cpool.tile([128, NG * 128], f32r)
            for b in range(NG // MB):
                l16 = ps_l.tile([128, MB * 128], f32, tag="l16")
                nc.tensor.matmul(l16[:], lhsT=wl4,
                                 rhs=xst[:, b * MB * 128:(b + 1) * MB * 128],
                                 start=True, stop=True)
                nc.scalar.activation(sst_all[:, b * MB * 128:(b + 1) * MB * 128],
                                     l16[:], ACTF.Exp, bias=negk4, scale=1.0)

            g0m = 0                 # first group of this meta
            pair_base = 0           # global pair index base
            for m, SZ in enumerate(METAS):
                GJ = SZ * TPG       # tiles (and S-columns) in this meta
                sS = ps_s.tile([128, R], f32, tag="sS")
                # seed sS with eps: ones128^T @ epscol = eps everywhere
                nc.tensor.matmul(sS[:, 0:GJ], lhsT=ones128[:],
                                 rhs=epscol[:, 0:GJ],
                                 start=True, stop=False, skip_group_check=True)
                u32 = mpool.tile([128, META * TPG * O], f32, tag="u32")

                for P in range(SZ // 2):        # group pairs
                    JP = 2 * TPG                # 8 tiles per pair
                    gs = wpool.tile([128, JP * F], bf16, tag="gs")
                    for k in range(2):
                        q = P * 2 + k
                        g = g0m + q
                        sst_g = sst_all[:, g * 128:(g + 1) * 128]

                        g4 = ps_g.tile([128, TPG * FPAD], f32, tag="g4")
                        nc.tensor.matmul(g4[:, 0:512], lhsT=sst_g,
                                         rhs=c2d4[:, 0:512],
                                         start=True, stop=True)
                        nc.tensor.matmul(g4[:, 512:1024], lhsT=sst_g,
                                         rhs=c2d4[:, 512:1024],
                                         start=True, stop=True)
                        nc.tensor.matmul(sS[:, TPG * q:TPG * (q + 1)],
                                         lhsT=sst_g, rhs=srhs,
                                         start=False, stop=(q == SZ - 1),
                                         skip_group_check=True)
                        # ACT: PSUM->SBUF gather of the used cols, cast bf16
                        nc.scalar.activation(
                            gs[:, k * TPG * F:(k + 1) * TPG * F].rearrange(
                                "p (j f) -> p j f", j=TPG),
                            g4[:].rearrange("p (j f) -> p j f",
                                            j=TPG)[:, :, 0:F],
                            ACTF.Copy)

                    # DVE 2x multiply for the pair: p4 = gs * xhat
                    g0 = g0m + P * 2
                    p4 = wpool.tile([128, JP * F], bf16, tag="p4")
                    p4v = p4[:].rearrange("p (j o i) -> p j o i", j=JP, o=O)
                    gsv = gs[:].rearrange("p (j o i) -> p j o i", j=JP, o=O)
                    xhv = xrv[:, TPG * g0:TPG * (g0 + 2), :].unsqueeze(
                        2).broadcast_to([128, JP, O, DI])
                    nc.vector.tensor_tensor(p4v, gsv, xhv, ALU.mult)

                    # i-reduction -> u32[:, P*80:(P+1)*80]
                    uslice = u32[:, P * JP * O:(P + 1) * JP * O].rearrange(
                        "p (j o) -> p j o", j=JP)
                    p4jo = p4[:].rearrange("p (jo i) -> p jo i", i=DI)
                    if P + pair_base in POOL_RED:
                        t8 = wpool.tile([128, JP * O * 8], f32, tag="t8")
                        t8v = t8[:].rearrange("p (jo i) -> p jo i", i=8)
                        nc.gpsimd.tensor_tensor(
                            t8v, p4jo[:, :, 0:8], p4jo[:, :, 8:16], ALU.add)
                        t4 = wpool.tile([128, JP * O * 4], f32, tag="t4")
                        t4v = t4[:].rearrange("p (jo i) -> p jo i", i=4)
                        nc.gpsimd.tensor_tensor(
                            t4v, t8v[:, :, 0:4], t8v[:, :, 4:8], ALU.add)
                        t2 = wpool.tile([128, JP * O * 2], f32, tag="t2")
                        t2v = t2[:].rearrange("p (jo i) -> p jo i", i=2)
                        nc.gpsimd.tensor_tensor(
                            t2v, t4v[:, :, 0:2], t4v[:, :, 2:4], ALU.add)
                        t1 = wpool.tile([128, JP * O], f32, tag="t1")
                        nc.gpsimd.tensor_tensor(
                            t1[:],
                            t2v[:, :, 0:1].rearrange("p a b -> p (a b)"),
                            t2v[:, :, 1:2].rearrange("p a b -> p (a b)"),
                            ALU.add)
                        nc.gpsimd.tensor_tensor(
                            uslice.rearrange("p a b -> p (a b)"), t1[:],
                            p4jo[:, :, 16:17].rearrange("p a b -> p (a b)"),
                            ALU.add)
                    else:
                        nc.vector.tensor_reduce(uslice, p4jo,
                                                axis=AX.X, op=ALU.add)

                # ---- per-meta softmax ------------------------------------
                s32r = mpool.tile([128, R], f32, tag="s32r")
                nc.vector.reciprocal(s32r[:, 0:GJ], sS[:, 0:GJ])
                uv = u32[:, 0:GJ * O]
                u32v = uv.rearrange("p (g o) -> p g o", o=O)
                s32b = s32r[:, 0:GJ].unsqueeze(2).broadcast_to([128, GJ, O])
                eng_un = nc.gpsimd if SZ > 4 else nc.vector
                eng_un.tensor_tensor(u32v, u32v, s32b, ALU.mult)
                e32 = mpool.tile([128, META * TPG * O], f32, tag="e32")
                ev = e32[:, 0:GJ * O].rearrange("p (g o) -> p g o", o=O)
                nc.scalar.activation(e32[:, 0:GJ * O], uv, ACTF.Exp)
                se32 = mpool.tile([128, R], f32, tag="se32")
                if m in POOL_SE:
                    t5 = mpool.tile([128, R * 5], f32, tag="set5")
                    t5v = t5[:, 0:GJ * 5].rearrange("p (g o) -> p g o", o=5)
                    nc.gpsimd.tensor_tensor(t5v, ev[:, :, 0:5],
                                            ev[:, :, 5:10], ALU.add)
                    t21 = mpool.tile([128, R * 2], f32, tag="set2")
                    t21v = t21[:, 0:GJ * 2].rearrange("p (g o) -> p g o", o=2)
                    nc.gpsimd.tensor_tensor(t21v, t5v[:, :, 0:2],
                                            t5v[:, :, 2:4], ALU.add)
                    tse = mpool.tile([128, R], f32, tag="setse")
                    nc.gpsimd.tensor_tensor(
                        tse[:, 0:GJ],
                        t21v[:, :, 0:1].rearrange("p a b -> p (a b)"),
                        t21v[:, :, 1:2].rearrange("p a b -> p (a b)"),
                        ALU.add)
                    nc.gpsimd.tensor_tensor(
                        se32[:, 0:GJ], tse[:, 0:GJ],
                        t5v[:, :, 4:5].rearrange("p a b -> p (a b)"),
                        ALU.add)
                else:
                    nc.vector.tensor_reduce(se32[:, 0:GJ], ev,
                                            axis=AX.X, op=ALU.add)
                nc.vector.reciprocal(se32[:, 0:GJ], se32[:, 0:GJ])
                se32b = se32[:, 0:GJ].unsqueeze(2).broadcast_to([128, GJ, O])
                o_all = mpool.tile([128, META * TPG * O], f32, tag="oall")
                nc.vector.tensor_tensor(
                    o_all[:, 0:GJ * O].rearrange("p (g o) -> p g o", o=O),
                    ev, se32b, ALU.mult)
                nc.sync.dma_start(
                    out=youtv[:, TPG * g0m:TPG * (g0m + SZ), :],
                    in_=o_all[:, 0:GJ * O].rearrange("p (t o) -> p t o", o=O))
                g0m += SZ
                pair_base += SZ // 2
    nc.compile()
    return nc


def _pack(Xc):
    """Per-core host packing.

    xst (128, 4096) f32: partition (j,s,d) = j*32+s*16+d, col g*128+p,
        value x[n,d]^(s+1) for row n = p*128 + (4g+j).
    xrb (128, 2176) bf16: [p, t*17+i] = xhat for row n = p*128+t.
    """
    import ml_dtypes
    X3 = Xc.reshape(128, NT, D)                       # [p, t, d]
    A = X3.transpose(1, 2, 0)                         # [t, d, p]
    G4v = A.reshape(NG, TPG, D, 128)                  # [g, j, d, p]
    B = np.stack([G4v, G4v * G4v], axis=2)            # [g, j, s, d, p]
    xst = np.ascontiguousarray(
        B.transpose(1, 2, 3, 0, 4).reshape(128, NG * 128))
    xr = np.concatenate([X3, np.ones((128, NT, 1), np.float32)], axis=2)
    xrb = np.ascontiguousarray(xr.reshape(128, NT * DI)).astype(np.float16)
    return xst, xrb


_NC_CACHE = None


def kernel(X, centers, sigmas, coeffs):
    global _NC_CACHE
    from concourse import bass_utils

    X = np.asarray(X, np.float32)
    cst, c2d4 = _build_constants(
        np.asarray(centers, np.float32),
        np.asarray(sigmas, np.float32),
        np.asarray(coeffs, np.float32))

    if _NC_CACHE is None:
        _NC_CACHE = _build_bass()
    nc = _NC_CACHE

    in_maps = []
    for c in range(NCORES):
        xst, xrb = _pack(X[c * MC:(c + 1) * MC])
        in_maps.append({"xst": xst, "xrb": xrb, "cst": cst, "c2d4": c2d4})
    res = bass_utils.run_bass_kernel_spmd(nc, in_maps, list(range(NCORES)))
    return np.concatenate([r["yout"] for r in res.results], axis=0)


# revision 35
# speedup vs baseline: 1.5215x; 1.1120x over previous
"""ANFIS forward kernel for Trainium2, 8-core data-parallel. v7.

Algebra per row n (see reference):
    l_r = sum_d [2*c*a*x - a*x^2] - k_r;  s_r = exp(l_r)
    G_f = sum_r s_r * Chat[r,f]   (f = (o,i) products, i innermost)
    U_o = sum_i xhat_i * G_(o,i);  out = softmax_o(U / (S + eps))

v7 changes vs v6:
  - Host supplies the PE-transpose layout directly (xst), killing the
    on-chip transposes and their PSUM->SBUF copies.
  - M1 is one f32r matmul per 4-group batch (256+ wide -> 1 cycle/row).
  - S+eps comes from tiny per-group PE matmuls accumulated into a
    per-meta PSUM tile (eps seeded by a ones x eps/128 matmul), so the
    32 per-group ACT extracts become one DVE reciprocal per meta.
  - G is copied PSUM->SBUF by the otherwise-idle ACT engine with a cast
    to bf16; the big per-row multiply then runs in the DVE 2x_1p mode,
    and the i-reduction is split between DVE tensor_reduce and Pool
    tensor_tensor fold chains (Pool cannot touch PSUM, but p4 is SBUF).
  - Per-meta output stores instead of one tail store.

Layout: row n of a core's 16384-row slice -> (p, t) = (n // 128, n % 128).
Group g = tiles [4g, 4g+4); meta m = groups [8m, 8m+8) = tiles [32m, 32m+32).
"""

import numpy as np

N, D, R, O = 131072, 16, 32, 10
EPS = 1e-8
NCORES = 8
MC = N // NCORES          # rows per core = 16384
TPG = 4                   # tiles (of 128 rows) per group
NG = 32                   # groups per core
MB = 4                    # groups per M1 batch
META = 8                  # groups per softmax batch
NMETA = NG // META        # 4

DI = D + 1                # 17: x dims + ones
F = O * DI                # 170 product features, f = o*17 + i
FPAD = 256                # per-j feature stride in g4 (bank alignment)
NT = MC // 128            # 128 tiles per core

# input chunks in groups: early small chunk -> early compute start
CH = [(0, 4), (4, 4), (8, 8), (16, 16)]

# softmax batches, in groups; smaller ones at the end shorten the drain
METAS = [8, 8, 8, 4, 4]

# group PAIRS whose i-reduction runs as a Pool fold chain (rest: DVE
# reduce). None in the last two metas: Pool drains slowly at the tail.
POOL_RED = frozenset({0, 1, 4, 5, 8, 9, 12, 14})
# pairs that skip the ACT copy: DVE multiplies fp32 straight from PSUM.
# Used at the head where DVE is otherwise idle and every ACT slot counts.
DIRECT = frozenset({0})
# metas whose se-reduce runs on Pool (fold) instead of DVE
POOL_SE = frozenset()


def _build_constants(centers, sigmas, coeffs):
    a = 1.0 / (2.0 * sigmas.astype(np.float64) ** 2)          # (R,D)
    c = centers.astype(np.float64)

    # WL4: lhsT for M1. out partition (j,r) = j*32+r; input partition
    # (j,s,d) = j*32 + s*16 + d  (s=0: x, s=1: x^2).
    wl4 = np.zeros((128, 128), np.float64)
    for j in range(TPG):
        for r in range(R):
            pi = j * R + r
            for d in range(D):
                wl4[j * 32 + 0 * 16 + d, pi] = 2.0 * c[r, d] * a[r, d]   # x
                wl4[j * 32 + 1 * 16 + d, pi] = -a[r, d]                  # x^2
    negk = -(c * c * a).sum(axis=1)                            # (R,)
    negk4 = np.tile(negk, TPG).reshape(128, 1)

    # Chat (R, 170): f = o*17+i (i=16 -> bias row)
    chat = coeffs.astype(np.float64).transpose(0, 2, 1).reshape(R, F)
    # C2D4 (128, 1024): [(j,r), j'*256+f] = delta_jj' * chat[r,f]
    c2d4 = np.zeros((128, TPG * FPAD), np.float64)
    for j in range(TPG):
        c2d4[j * R:(j + 1) * R, j * FPAD:j * FPAD + F] = chat
    # sS rhs (128, 4): [(j,r), j'] = delta_jj' -> S_j per group
    srhs = np.zeros((128, TPG), np.float64)
    for j in range(TPG):
        srhs[j * R:(j + 1) * R, j] = 1.0
    cst = np.concatenate([negk4, wl4, srhs], axis=1)           # (128, 133)
    return cst.astype(np.float32), c2d4.astype(np.float32)


def _build_bass():
    import concourse.bacc as bacc
    import concourse.mybir as mybir
    from concourse.tile import TileContext

    f32 = mybir.dt.float32
    f32r = mybir.dt.float32r
    bf16 = mybir.dt.bfloat16
    AX = mybir.AxisListType
    ALU = mybir.AluOpType
    ACTF = mybir.ActivationFunctionType

    nc = bacc.Bacc("TRN2", target_bir_lowering=False, debug=False)
    xst_d = nc.declare_dram_parameter("xst", [128, NG * 128], f32r,
                                      isOutput=False)
    fp16 = mybir.dt.float16
    xrb_d = nc.declare_dram_parameter("xrb", [128, NT * DI], fp16,
                                      isOutput=False)
    cst_d = nc.declare_dram_parameter("cst", [128, 133], f32r, isOutput=False)
    c2d4_d = nc.declare_dram_parameter("c2d4", [128, TPG * FPAD], f32r,
                                       isOutput=False)
    yout = nc.declare_dram_parameter("yout", [MC, O], f32, isOutput=True)

    youtv = yout[:, :].rearrange("(p t) o -> p t o", p=128)

    with TileContext(nc) as tc:
        with (
            tc.tile_pool(name="const", bufs=1) as cpool,
            tc.tile_pool(name="sst", bufs=2) as epool,
            tc.tile_pool(name="work", bufs=4) as wpool,
            tc.tile_pool(name="meta", bufs=3) as mpool,
            tc.tile_pool(name="ps_l", bufs=1, space="PSUM") as ps_l,
            tc.tile_pool(name="ps_g", bufs=3, space="PSUM") as ps_g,
            tc.tile_pool(name="ps_s", bufs=1, space="PSUM") as ps_s,
        ):
            # constants: cst via the Pool SWDGE path (its descriptor-gen
            # doesn't hold HWDGE), xst chunk0 first in the HWDGE queue.
            cst = cpool.tile([128, 133], f32r)
            nc.gpsimd.dma_start(out=cst[:], in_=cst_d[:, :])
            negk4 = cst[:, 0:1]
            wl4 = cst[:, 1:129]
            srhs = cst[:, 129:133]
            xst = cpool.tile([128, NG * 128], f32r)
            xrb = cpool.tile([128, NT * DI], fp16)
            c2d4 = cpool.tile([128, TPG * FPAD], f32r)
            ones128 = cpool.tile([128, 128], f32)
            epscol = cpool.tile([128, R], f32)
            nc.vector.memset(ones128[:], 1.0)
            nc.vector.memset(epscol[:], EPS / 128.0)
            # hoist the ACT exp-table load out of the critical path
            dummy = cpool.tile([128, 1], f32)
            nc.scalar.activation(dummy[:], epscol[:, 0:1], ACTF.Exp)
            for i, (s, n) in enumerate(CH):
                nc.sync.dma_start(out=xst[:, s * 128:(s + n) * 128],
                                  in_=xst_d[:, s * 128:(s + n) * 128])
                nc.sync.dma_start(out=xrb[:, s * TPG * DI:(s + n) * TPG * DI],
                                  in_=xrb_d[:, s * TPG * DI:(s + n) * TPG * DI])
                if i == 0:
                    nc.sync.dma_start(out=c2d4[:, 0:512],
                                      in_=c2d4_d[:, 0:512])
                    nc.sync.dma_start(out=c2d4[:, 512:1024],
                                      in_=c2d4_d[:, 512:1024])
            xrv = xrb[:].rearrange("p (t c) -> p t c", c=DI)

            # ---- phase 1: all M1 batches + exps -> sst_all in SBUF -------
            # Keeps the steady-state ACT stream homogeneous (copies only):
            # an exp interleaved between copies stalls the whole in-order
            # ACT queue on its M1 dependency.
            sst_all = cpool.tile([128, NG * 128], f32r)
            for b in range(NG // MB):
                l16 = ps_l.tile([128, MB * 128], f32, tag="l16")
                for h in range(0, MB * 128, 512):
                    nc.tensor.matmul(
                        l16[:, h:h + 512], lhsT=wl4,
                        rhs=xst[:, b * MB * 128 + h:b * MB * 128 + h + 512],
                        start=True, stop=True)
                nc.scalar.activation(sst_all[:, b * MB * 128:(b + 1) * MB * 128],
                                     l16[:], ACTF.Exp, bias=negk4, scale=1.0)

            g0m = 0                 # first group of this meta
            pair_base = 0           # global pair index base
            pending_tail = None     # deferred softmax-tail emitter
            for m, SZ in enumerate(METAS):
                GJ = SZ * TPG       # tiles (and S-columns) in this meta
                sS = ps_s.tile([128, R], f32, tag="sS")
                # seed sS with eps: ones128^T @ epscol = eps everywhere
                nc.tensor.matmul(sS[:, 0:GJ], lhsT=ones128[:],
                                 rhs=epscol[:, 0:GJ],
                                 start=True, stop=False, skip_group_check=True)
                u32 = mpool.tile([128, META * TPG * O], f32, tag="u32")

                for P in range(SZ // 2):        # group pairs
                    if P == 1 and pending_tail is not None:
                        # emit the previous meta's softmax tail here so its
                        # e32 exp doesn't head-of-line-block this meta's
                        # copies in the in-order ACT queue
                        pending_tail()
                        pending_tail = None
                    JP = 2 * TPG                # 8 tiles per pair
                    direct = (P + pair_base) in DIRECT
                    if direct:
                        p4 = wpool.tile([128, JP * F], f32, tag="p4f")
                    else:
                        p4 = wpool.tile([128, JP * F], bf16, tag="p4")
                        gs = wpool.tile([128, JP * F], bf16, tag="gs")
                    for k in range(2):
                        q = P * 2 + k
                        g = g0m + q
                        sst_g = sst_all[:, g * 128:(g + 1) * 128]

                        g4 = ps_g.tile([128, TPG * FPAD], f32, tag="g4")
                        nc.tensor.matmul(g4[:, 0:512], lhsT=sst_g,
                                         rhs=c2d4[:, 0:512],
                                         start=True, stop=True)
                        nc.tensor.matmul(g4[:, 512:1024], lhsT=sst_g,
                                         rhs=c2d4[:, 512:1024],
                                         start=True, stop=True)
                        nc.tensor.matmul(sS[:, TPG * q:TPG * (q + 1)],
                                         lhsT=sst_g, rhs=srhs,
                                         start=False, stop=(q == SZ - 1),
                                         skip_group_check=True)
                        g4f = g4[:].rearrange("p (j f) -> p j f",
                                              j=TPG)[:, :, 0:F]
                        if direct:
                            # DVE fp32 multiply straight from PSUM
                            xh1 = xrv[:, TPG * g:TPG * (g + 1), :].unsqueeze(
                                2).broadcast_to([128, TPG, O, DI])
                            nc.vector.tensor_tensor(
                                p4[:, k * TPG * F:(k + 1) * TPG * F].rearrange(
                                    "p (j o i) -> p j o i", j=TPG, o=O),
                                g4f.rearrange("p j (o i) -> p j o i", o=O),
                                xh1, ALU.mult)
                        else:
                            # ACT: PSUM->SBUF gather of used cols, cast bf16
                            nc.scalar.activation(
                                gs[:, k * TPG * F:(k + 1) * TPG * F].rearrange(
                                    "p (j f) -> p j f", j=TPG),
                                g4f, ACTF.Copy)

                    g0 = g0m + P * 2
                    if not direct:
                        # DVE 2x multiply for the pair: p4 = gs * xhat
                        p4v = p4[:].rearrange("p (j o i) -> p j o i",
                                              j=JP, o=O)
                        gsv = gs[:].rearrange("p (j o i) -> p j o i",
                                              j=JP, o=O)
                        xhv = xrv[:, TPG * g0:TPG * (g0 + 2), :].unsqueeze(
                            2).broadcast_to([128, JP, O, DI])
                        nc.vector.tensor_tensor(p4v, gsv, xhv, ALU.mult)

                    # i-reduction -> u32[:, P*80:(P+1)*80]
                    uslice = u32[:, P * JP * O:(P + 1) * JP * O].rearrange(
                        "p (j o) -> p j o", j=JP)
                    p4jo = p4[:].rearrange("p (jo i) -> p jo i", i=DI)
                    if P + pair_base in POOL_RED:
                        t8 = wpool.tile([128, JP * O * 8], f32, tag="t8")
                        t8v = t8[:].rearrange("p (jo i) -> p jo i", i=8)
                        nc.gpsimd.tensor_tensor(
                            t8v, p4jo[:, :, 0:8], p4jo[:, :, 8:16], ALU.add)
                        t4 = wpool.tile([128, JP * O * 4], f32, tag="t4")
                        t4v = t4[:].rearrange("p (jo i) -> p jo i", i=4)
                        nc.gpsimd.tensor_tensor(
                            t4v, t8v[:, :, 0:4], t8v[:, :, 4:8], ALU.add)
                        t2 = wpool.tile([128, JP * O * 2], f32, tag="t2")
                        t2v = t2[:].rearrange("p (jo i) -> p jo i", i=2)
                        nc.gpsimd.tensor_tensor(
                            t2v, t4v[:, :, 0:2], t4v[:, :, 2:4], ALU.add)
                        t1 = wpool.tile([128, JP * O], f32, tag="t1")
                        nc.gpsimd.tensor_tensor(
                            t1[:],
                            t2v[:, :, 0:1].rearrange("p a b -> p (a b)"),
                            t2v[:, :, 1:2].rearrange("p a b -> p (a b)"),
                            ALU.add)
                        nc.gpsimd.tensor_tensor(
                            uslice.rearrange("p a b -> p (a b)"), t1[:],
                            p4jo[:, :, 16:17].rearrange("p a b -> p (a b)"),
                            ALU.add)
                    else:
                        nc.vector.tensor_reduce(uslice, p4jo,
                                                axis=AX.X, op=ALU.add)

                def make_tail(m=m, SZ=SZ, GJ=GJ, g0m=g0m, sS=sS, u32=u32):
                    def tail():
                        s32r = mpool.tile([128, R], f32, tag="s32r")
                        nc.vector.reciprocal(s32r[:, 0:GJ], sS[:, 0:GJ])
                        uv = u32[:, 0:GJ * O]
                        u32v = uv.rearrange("p (g o) -> p g o", o=O)
                        s32b = s32r[:, 0:GJ].unsqueeze(2).broadcast_to(
                            [128, GJ, O])
                        nc.vector.tensor_tensor(u32v, u32v, s32b, ALU.mult)
                        e32 = mpool.tile([128, META * TPG * O], f32,
                                         tag="e32")
                        ev = e32[:, 0:GJ * O].rearrange("p (g o) -> p g o",
                                                        o=O)
                        nc.scalar.activation(e32[:, 0:GJ * O], uv, ACTF.Exp)
                        se32 = mpool.tile([128, R], f32, tag="se32")
                        nc.vector.tensor_reduce(se32[:, 0:GJ], ev,
                                                axis=AX.X, op=ALU.add)
                        nc.vector.reciprocal(se32[:, 0:GJ], se32[:, 0:GJ])
                        se32b = se32[:, 0:GJ].unsqueeze(2).broadcast_to(
                            [128, GJ, O])
                        o_all = mpool.tile([128, META * TPG * O], f32,
                                           tag="oall")
                        nc.vector.tensor_tensor(
                            o_all[:, 0:GJ * O].rearrange("p (g o) -> p g o",
                                                         o=O),
                            ev, se32b, ALU.mult)
                        nc.sync.dma_start(
                            out=youtv[:, TPG * g0m:TPG * (g0m + SZ), :],
                            in_=o_all[:, 0:GJ * O].rearrange(
                                "p (t o) -> p t o", o=O))
                    return tail

                pending_tail = make_tail()
                g0m += SZ
                pair_base += SZ // 2
            pending_tail()
    nc.compile()
    return nc


def _pack(Xc):
    """Per-core host packing.

    xst (128, 4096) f32: partition (j,s,d) = j*32+s*16+d, col g*128+p,
        value x[n,d]^(s+1) for row n = p*128 + (4g+j).
    xrb (128, 2176) bf16: [p, t*17+i] = xhat for row n = p*128+t.
    """
    import ml_dtypes
    X3 = Xc.reshape(128, NT, D)                       # [p, t, d]
    A = X3.transpose(1, 2, 0)                         # [t, d, p]
    G4v = A.reshape(NG, TPG, D, 128)                  # [g, j, d, p]
    B = np.stack([G4v, G4v * G4v], axis=2)            # [g, j, s, d, p]
    xst = np.ascontiguousarray(
        B.transpose(1, 2, 3, 0, 4).reshape(128, NG * 128))
    xr = np.concatenate([X3, np.ones((128, NT, 1), np.float32)], axis=2)
    xrb = np.ascontiguousarray(xr.reshape(128, NT * DI)).astype(np.float16)
    return xst, xrb


_NC_CACHE = None


def kernel(X, centers, sigmas, coeffs):
    global _NC_CACHE
    from concourse import bass_utils

    X = np.asarray(X, np.float32)
    cst, c2d4 = _build_constants(
        np.asarray(centers, np.float32),
        np.asarray(sigmas, np.float32),
        np.asarray(coeffs, np.float32))

    if _NC_CACHE is None:
        _NC_CACHE = _build_bass()
    nc = _NC_CACHE

    in_maps = []
    for c in range(NCORES):
        xst, xrb = _pack(X[c * MC:(c + 1) * MC])
        in_maps.append({"xst": xst, "xrb": xrb, "cst": cst, "c2d4": c2d4})
    res = bass_utils.run_bass_kernel_spmd(nc, in_maps, list(range(NCORES)))
    return np.concatenate([r["yout"] for r in res.results], axis=0)


# revision 86
# speedup vs baseline: 1.6592x; 1.0905x over previous
"""ANFIS forward kernel for Trainium2, 8-core data-parallel. v8.

Algebra per row n (see reference):
    l_r = sum_d [2*c*a*x - a*x^2] - k_r;  s_r = exp(l_r)
    G_f = sum_r s_r * Chat[r,f]   (f = (o,i) products, i innermost)
    U_o = sum_i xhat_i * G_(o,i);  out = softmax_o(U / (S + eps))

Design (v8, 41.5us vs 68.7us v6 baseline under the TimelineSim model):
  - Host supplies the PE-transpose layout directly (xst f32r) and xhat
    in fp16 (xrb), killing all on-chip transposes.
  - Phase 1 computes every M1 batch (one 512-wide f32r matmul, 1 c/row)
    plus its ACT exp up front into sst_all, so the steady-state ACT
    queue is homogeneous copies (an exp interleaved between copies
    head-of-line-blocks the in-order ACT sequencer on its M1 dep).
  - M2 per group: two 512-col f32r matmuls (FPAD=256 bank layout) plus
    a 4-col S-matmul accumulating S into a per-meta PSUM tile seeded
    with eps by a ones128 @ (eps/128) matmul; one DVE reciprocal per
    meta replaces 32 per-group ACT extracts.
  - ACT copies G PSUM->SBUF with a cast to bf16 (680 cols, 752ns); the
    per-row multiply then runs on DVE in 2x_1p mode over group PAIRS
    (all-16-bit operands, i innermost packed; fp16 xhat is free
    accuracy); DIRECT pairs instead multiply fp32 straight from PSUM so
    the pipeline head does not wait for ACT.
  - i-reduction per pair: a DVE bf16 fold chain (the first three folds
    hit the 2x_1p mode, which tensor_reduce never can: 1049 vs 1477ns),
    or a Pool fp32 tensor_tensor fold chain for POOL_RED pairs (Pool
    cannot touch PSUM, but p4 is SBUF; Pool has no 2x so fp32 is free
    accuracy there). POOL_RED interleaved so neither engine backs up.
  - Softmax per meta (METAS sizes, small ones last to shorten the
    drain); each meta's tail is emitted one pair into the next meta so
    its e32 exp does not block the next meta's copies; per-meta stores.

Layout: row n of a core's 16384-row slice -> (p, t) = (n // 128, n % 128).
Group g = tiles [4g, 4g+4); metas cover METAS[i] groups each.
"""

import numpy as np

N, D, R, O = 131072, 16, 32, 10
EPS = 1e-8
NCORES = 8
MC = N // NCORES          # rows per core = 16384
TPG = 4                   # tiles (of 128 rows) per group
NG = 32                   # groups per core
MB = 4                    # groups per M1 batch
META = 8                  # groups per softmax batch
NMETA = NG // META        # 4

DI = D + 1                # 17: x dims + ones
F = O * DI                # 170 product features, f = o*17 + i
FPAD = 256                # per-j feature stride in g4 (bank alignment)
NT = MC // 128            # 128 tiles per core

# input chunks in groups: early small chunk -> early compute start
CH = [(0, 4), (4, 4), (8, 8), (16, 16)]

# softmax batches, in groups; smaller ones at the end shorten the drain
METAS = [4, 8, 8, 8, 4]

# group PAIRS whose i-reduction runs as a Pool fold chain (rest: DVE
# reduce). None in the last two metas: Pool drains slowly at the tail.
POOL_RED = frozenset({0, 4, 6, 8, 10, 12, 14})
# pairs that skip the ACT copy: DVE multiplies fp32 straight from PSUM.
# Used at the head where DVE is otherwise idle and every ACT slot counts.
DIRECT = frozenset({0, 4})
# pairs whose reduce is split: group A on Pool chain, group B on DVE
SPLIT_RED = frozenset()
# metas whose se-reduce runs on Pool (fold) instead of DVE
POOL_SE = frozenset()


def _build_constants(centers, sigmas, coeffs):
    a = 1.0 / (2.0 * sigmas.astype(np.float64) ** 2)          # (R,D)
    c = centers.astype(np.float64)

    # WL4: lhsT for M1. out partition (j,r) = j*32+r; input partition
    # (j,s,d) = j*32 + s*16 + d  (s=0: x, s=1: x^2).
    wl4 = np.zeros((128, 128), np.float64)
    for j in range(TPG):
        for r in range(R):
            pi = j * R + r
            for d in range(D):
                wl4[j * 32 + 0 * 16 + d, pi] = 2.0 * c[r, d] * a[r, d]   # x
                wl4[j * 32 + 1 * 16 + d, pi] = -a[r, d]                  # x^2
    negk = -(c * c * a).sum(axis=1)                            # (R,)
    negk4 = np.tile(negk, TPG).reshape(128, 1)

    # Chat (R, 170): f = o*17+i (i=16 -> bias row)
    chat = coeffs.astype(np.float64).transpose(0, 2, 1).reshape(R, F)
    # C2D4 (128, 1024): [(j,r), j'*256+f] = delta_jj' * chat[r,f]
    c2d4 = np.zeros((128, TPG * FPAD), np.float64)
    for j in range(TPG):
        c2d4[j * R:(j + 1) * R, j * FPAD:j * FPAD + F] = chat
    # sS rhs (128, 4): [(j,r), j'] = delta_jj' -> S_j per group
    srhs = np.zeros((128, TPG), np.float64)
    for j in range(TPG):
        srhs[j * R:(j + 1) * R, j] = 1.0
    cst = np.concatenate([negk4, wl4, srhs], axis=1)           # (128, 133)
    return cst.astype(np.float32), c2d4.astype(np.float32)


def _build_bass():
    import concourse.bacc as bacc
    import concourse.mybir as mybir
    from concourse.tile import TileContext

    f32 = mybir.dt.float32
    f32r = mybir.dt.float32r
    bf16 = mybir.dt.bfloat16
    AX = mybir.AxisListType
    ALU = mybir.AluOpType
    ACTF = mybir.ActivationFunctionType

    nc = bacc.Bacc("TRN2", target_bir_lowering=False, debug=False)
    xst_d = nc.declare_dram_parameter("xst", [128, NG * 128], f32r,
                                      isOutput=False)
    fp16 = mybir.dt.float16
    xrb_d = nc.declare_dram_parameter("xrb", [128, NT * DI], fp16,
                                      isOutput=False)
    cst_d = nc.declare_dram_parameter("cst", [128, 133], f32r, isOutput=False)
    c2d4_d = nc.declare_dram_parameter("c2d4", [128, TPG * FPAD], f32r,
                                       isOutput=False)
    yout = nc.declare_dram_parameter("yout", [MC, O], f32, isOutput=True)

    youtv = yout[:, :].rearrange("(p t) o -> p t o", p=128)

    with TileContext(nc) as tc:
        with (
            tc.tile_pool(name="const", bufs=1) as cpool,
            tc.tile_pool(name="work", bufs=6) as wpool,
            tc.tile_pool(name="meta", bufs=4) as mpool,
            tc.tile_pool(name="ps_l", bufs=1, space="PSUM") as ps_l,
            tc.tile_pool(name="ps_g", bufs=3, space="PSUM") as ps_g,
            tc.tile_pool(name="ps_s", bufs=1, space="PSUM") as ps_s,
        ):
            # constants: cst via the Pool SWDGE path (its descriptor-gen
            # doesn't hold HWDGE), xst chunk0 first in the HWDGE queue.
            cst = cpool.tile([128, 133], f32r)
            nc.gpsimd.dma_start(out=cst[:], in_=cst_d[:, :])
            negk4 = cst[:, 0:1]
            wl4 = cst[:, 1:129]
            srhs = cst[:, 129:133]
            xst = cpool.tile([128, NG * 128], f32r)
            xrb = cpool.tile([128, NT * DI], fp16)
            c2d4 = cpool.tile([128, TPG * FPAD], f32r)
            ones128 = cpool.tile([128, 128], f32)
            epscol = cpool.tile([128, R], f32)
            nc.vector.memset(ones128[:], 1.0)
            nc.vector.memset(epscol[:], EPS / 128.0)
            # hoist the ACT exp-table load out of the critical path
            dummy = cpool.tile([128, 1], f32)
            nc.scalar.activation(dummy[:], epscol[:, 0:1], ACTF.Exp)
            for i, (s, n) in enumerate(CH):
                nc.sync.dma_start(out=xst[:, s * 128:(s + n) * 128],
                                  in_=xst_d[:, s * 128:(s + n) * 128])
                nc.sync.dma_start(out=xrb[:, s * TPG * DI:(s + n) * TPG * DI],
                                  in_=xrb_d[:, s * TPG * DI:(s + n) * TPG * DI])
                if i == 0:
                    nc.sync.dma_start(out=c2d4[:, 0:512],
                                      in_=c2d4_d[:, 0:512])
                    nc.sync.dma_start(out=c2d4[:, 512:1024],
                                      in_=c2d4_d[:, 512:1024])
            xrv = xrb[:].rearrange("p (t c) -> p t c", c=DI)

            # ---- phase 1: all M1 batches + exps -> sst_all in SBUF -------
            # Keeps the steady-state ACT stream homogeneous (copies only):
            # an exp interleaved between copies stalls the whole in-order
            # ACT queue on its M1 dependency.
            sst_all = cpool.tile([128, NG * 128], f32r)
            gb = 0
            for nb in [4] * 8:
                w = nb * 128
                l16 = ps_l.tile([128, MB * 128], f32, tag="l16")
                for h in range(0, w, 512):
                    nc.tensor.matmul(
                        l16[:, h:min(h + 512, w)], lhsT=wl4,
                        rhs=xst[:, gb * 128 + h:gb * 128 + min(h + 512, w)],
                        start=True, stop=True)
                nc.scalar.activation(sst_all[:, gb * 128:gb * 128 + w],
                                     l16[:, 0:w], ACTF.Exp,
                                     bias=negk4, scale=1.0)
                gb += nb

            g0m = 0                 # first group of this meta
            pair_base = 0           # global pair index base
            pending_tail = None     # deferred softmax-tail emitter
            for m, SZ in enumerate(METAS):
                GJ = SZ * TPG       # tiles (and S-columns) in this meta
                sS = ps_s.tile([128, R], f32, tag="sS")
                # seed sS with eps: ones128^T @ epscol = eps everywhere
                nc.tensor.matmul(sS[:, 0:GJ], lhsT=ones128[:],
                                 rhs=epscol[:, 0:GJ],
                                 start=True, stop=False, skip_group_check=True)
                u32 = mpool.tile([128, META * TPG * O], f32, tag="u32")

                last_meta = (m == len(METAS) - 1)
                for P in range(SZ // 2):        # group pairs
                    if (P == min(1, SZ // 2 - 1) and not last_meta
                            and pending_tail is not None):
                        # emit the previous meta's softmax tail here so its
                        # e32 exp doesn't head-of-line-block this meta's
                        # copies in the in-order ACT queue
                        pending_tail()
                        pending_tail = None
                    JP = 2 * TPG                # 8 tiles per pair
                    direct = (P + pair_base) in DIRECT
                    if direct:
                        p4 = wpool.tile([128, JP * F], f32, tag="p4f")
                    else:
                        p4 = wpool.tile([128, JP * F], bf16, tag="p4")
                        gs = wpool.tile([128, JP * F], bf16, tag="gs")
                    for k in range(2):
                        q = P * 2 + k
                        g = g0m + q
                        sst_g = sst_all[:, g * 128:(g + 1) * 128]

                        g4 = ps_g.tile([128, TPG * FPAD], f32, tag="g4")
                        nc.tensor.matmul(g4[:, 0:512], lhsT=sst_g,
                                         rhs=c2d4[:, 0:512],
                                         start=True, stop=True)
                        nc.tensor.matmul(g4[:, 512:1024], lhsT=sst_g,
                                         rhs=c2d4[:, 512:1024],
                                         start=True, stop=True)
                        nc.tensor.matmul(sS[:, TPG * q:TPG * (q + 1)],
                                         lhsT=sst_g, rhs=srhs,
                                         start=False, stop=(q == SZ - 1),
                                         skip_group_check=True)
                        g4f = g4[:].rearrange("p (j f) -> p j f",
                                              j=TPG)[:, :, 0:F]
                        if direct:
                            # DVE fp32 multiply straight from PSUM
                            xh1 = xrv[:, TPG * g:TPG * (g + 1), :].unsqueeze(
                                2).broadcast_to([128, TPG, O, DI])
                            nc.vector.tensor_tensor(
                                p4[:, k * TPG * F:(k + 1) * TPG * F].rearrange(
                                    "p (j o i) -> p j o i", j=TPG, o=O),
                                g4f.rearrange("p j (o i) -> p j o i", o=O),
                                xh1, ALU.mult)
                        else:
                            # ACT: PSUM->SBUF gather of used cols, cast bf16
                            nc.scalar.activation(
                                gs[:, k * TPG * F:(k + 1) * TPG * F].rearrange(
                                    "p (j f) -> p j f", j=TPG),
                                g4f, ACTF.Copy)

                    g0 = g0m + P * 2
                    if not direct:
                        # DVE 2x multiply for the pair: p4 = gs * xhat
                        p4v = p4[:].rearrange("p (j o i) -> p j o i",
                                              j=JP, o=O)
                        gsv = gs[:].rearrange("p (j o i) -> p j o i",
                                              j=JP, o=O)
                        xhv = xrv[:, TPG * g0:TPG * (g0 + 2), :].unsqueeze(
                            2).broadcast_to([128, JP, O, DI])
                        nc.vector.tensor_tensor(p4v, gsv, xhv, ALU.mult)

                    # i-reduction -> u32[:, P*80:(P+1)*80]
                    uslice = u32[:, P * JP * O:(P + 1) * JP * O].rearrange(
                        "p (j o) -> p j o", j=JP)
                    p4jo = p4[:].rearrange("p (jo i) -> p jo i", i=DI)
                    if P + pair_base in SPLIT_RED:
                        half = TPG * O
                        nc.vector.tensor_reduce(
                            uslice[:, 0:TPG, :],
                            p4jo[:, 0:half, :], axis=AX.X, op=ALU.add)
                        pj = p4jo[:, half:2 * half, :]
                        us2 = u32[:, P * JP * O + half:(P + 1) * JP * O]
                        t8 = wpool.tile([128, JP * O * 8], f32, tag="t8")
                        t8v = t8[:, 0:half * 8].rearrange(
                            "p (jo i) -> p jo i", i=8)
                        nc.gpsimd.tensor_tensor(
                            t8v, pj[:, :, 0:8], pj[:, :, 8:16], ALU.add)
                        t4 = wpool.tile([128, JP * O * 4], f32, tag="t4")
                        t4v = t4[:, 0:half * 4].rearrange(
                            "p (jo i) -> p jo i", i=4)
                        nc.gpsimd.tensor_tensor(
                            t4v, t8v[:, :, 0:4], t8v[:, :, 4:8], ALU.add)
                        t2 = wpool.tile([128, JP * O * 2], f32, tag="t2")
                        t2v = t2[:, 0:half * 2].rearrange(
                            "p (jo i) -> p jo i", i=2)
                        nc.gpsimd.tensor_tensor(
                            t2v, t4v[:, :, 0:2], t4v[:, :, 2:4], ALU.add)
                        t1 = wpool.tile([128, JP * O], f32, tag="t1")
                        nc.gpsimd.tensor_tensor(
                            t1[:, 0:half],
                            t2v[:, :, 0:1].rearrange("p a b -> p (a b)"),
                            t2v[:, :, 1:2].rearrange("p a b -> p (a b)"),
                            ALU.add)
                        nc.gpsimd.tensor_tensor(
                            us2, t1[:, 0:half],
                            pj[:, :, 16:17].rearrange("p a b -> p (a b)"),
                            ALU.add)
                    elif P + pair_base in POOL_RED:
                        t8 = wpool.tile([128, JP * O * 8], f32, tag="t8")
                        t8v = t8[:].rearrange("p (jo i) -> p jo i", i=8)
                        nc.gpsimd.tensor_tensor(
                            t8v, p4jo[:, :, 0:8], p4jo[:, :, 8:16], ALU.add)
                        t4 = wpool.tile([128, JP * O * 4], f32, tag="t4")
                        t4v = t4[:].rearrange("p (jo i) -> p jo i", i=4)
                        nc.gpsimd.tensor_tensor(
                            t4v, t8v[:, :, 0:4], t8v[:, :, 4:8], ALU.add)
                        t2 = wpool.tile([128, JP * O * 2], f32, tag="t2")
                        t2v = t2[:].rearrange("p (jo i) -> p jo i", i=2)
                        nc.gpsimd.tensor_tensor(
                            t2v, t4v[:, :, 0:2], t4v[:, :, 2:4], ALU.add)
                        t1 = wpool.tile([128, JP * O], f32, tag="t1")
                        nc.gpsimd.tensor_tensor(
                            t1[:],
                            t2v[:, :, 0:1].rearrange("p a b -> p (a b)"),
                            t2v[:, :, 1:2].rearrange("p a b -> p (a b)"),
                            ALU.add)
                        nc.gpsimd.tensor_tensor(
                            uslice.rearrange("p a b -> p (a b)"), t1[:],
                            p4jo[:, :, 16:17].rearrange("p a b -> p (a b)"),
                            ALU.add)
                    else:
                        # DVE bf16 fold chain: the first three folds run in
                        # the 2x_1p mode (all-bf16 packed), which
                        # tensor_reduce can never use -> 1049ns vs 1477ns
                        d8 = wpool.tile([128, JP * O * 8], bf16, tag="d8")
                        d8v = d8[:].rearrange("p (jo i) -> p jo i", i=8)
                        nc.vector.tensor_tensor(
                            d8v, p4jo[:, :, 0:8], p4jo[:, :, 8:16], ALU.add)
                        d4 = wpool.tile([128, JP * O * 4], bf16, tag="d4")
                        d4v = d4[:].rearrange("p (jo i) -> p jo i", i=4)
                        nc.vector.tensor_tensor(
                            d4v, d8v[:, :, 0:4], d8v[:, :, 4:8], ALU.add)
                        d2 = wpool.tile([128, JP * O * 2], bf16, tag="d2")
                        d2v = d2[:].rearrange("p (jo i) -> p jo i", i=2)
                        nc.vector.tensor_tensor(
                            d2v, d4v[:, :, 0:2], d4v[:, :, 2:4], ALU.add)
                        d1 = wpool.tile([128, JP * O], f32, tag="d1")
                        nc.vector.tensor_tensor(
                            d1[:],
                            d2v[:, :, 0:1].rearrange("p a b -> p (a b)"),
                            d2v[:, :, 1:2].rearrange("p a b -> p (a b)"),
                            ALU.add)
                        nc.vector.tensor_tensor(
                            uslice.rearrange("p a b -> p (a b)"), d1[:],
                            p4jo[:, :, 16:17].rearrange("p a b -> p (a b)"),
                            ALU.add)

                def make_tail(m=m, SZ=SZ, GJ=GJ, g0m=g0m, sS=sS, u32=u32):
                    def tail():
                        s32r = mpool.tile([128, R], f32, tag="s32r")
                        nc.vector.reciprocal(s32r[:, 0:GJ], sS[:, 0:GJ])
                        uv = u32[:, 0:GJ * O]
                        u32v = uv.rearrange("p (g o) -> p g o", o=O)
                        s32b = s32r[:, 0:GJ].unsqueeze(2).broadcast_to(
                            [128, GJ, O])
                        eng_un = nc.gpsimd if m in POOL_SE else nc.vector
                        eng_un.tensor_tensor(u32v, u32v, s32b, ALU.mult)
                        e32 = mpool.tile([128, META * TPG * O], f32,
                                         tag="e32")
                        ev = e32[:, 0:GJ * O].rearrange("p (g o) -> p g o",
                                                        o=O)
                        nc.scalar.activation(e32[:, 0:GJ * O], uv, ACTF.Exp)
                        se32 = mpool.tile([128, R], f32, tag="se32")
                        nc.vector.tensor_reduce(se32[:, 0:GJ], ev,
                                                axis=AX.X, op=ALU.add)
                        nc.vector.reciprocal(se32[:, 0:GJ], se32[:, 0:GJ])
                        se32b = se32[:, 0:GJ].unsqueeze(2).broadcast_to(
                            [128, GJ, O])
                        o_all = mpool.tile([128, META * TPG * O], f32,
                                           tag="oall")
                        nc.vector.tensor_tensor(
                            o_all[:, 0:GJ * O].rearrange("p (g o) -> p g o",
                                                         o=O),
                            ev, se32b, ALU.mult)
                        nc.sync.dma_start(
                            out=youtv[:, TPG * g0m:TPG * (g0m + SZ), :],
                            in_=o_all[:, 0:GJ * O].rearrange(
                                "p (t o) -> p t o", o=O))
                    return tail

                if pending_tail is not None:
                    # last meta: flush the previous tail only after ALL its
                    # copies are emitted, so they aren't blocked behind it
                    pending_tail()
                pending_tail = make_tail()
                g0m += SZ
                pair_base += SZ // 2
            pending_tail()
    nc.compile()
    return nc


def _pack(Xc):
    """Per-core host packing.

    xst (128, 4096) f32: partition (j,s,d) = j*32+s*16+d, col g*128+p,
        value x[n,d]^(s+1) for row n = p*128 + (4g+j).
    xrb (128, 2176) fp16: [p, t*17+i] = xhat for row n = p*128+t.
    """
    X3 = Xc.reshape(128, NT, D)                       # [p, t, d]
    A = X3.transpose(1, 2, 0)                         # [t, d, p]
    G4v = A.reshape(NG, TPG, D, 128)                  # [g, j, d, p]
    B = np.stack([G4v, G4v * G4v], axis=2)            # [g, j, s, d, p]
    xst = np.ascontiguousarray(
        B.transpose(1, 2, 3, 0, 4).reshape(128, NG * 128))
    xr = np.concatenate([X3, np.ones((128, NT, 1), np.float32)], axis=2)
    xrb = np.ascontiguousarray(xr.reshape(128, NT * DI)).astype(np.float16)
    return xst, xrb


_NC_CACHE = None


def kernel(X, centers, sigmas, coeffs):
    global _NC_CACHE
    from concourse import bass_utils

    X = np.asarray(X, np.float32)
    cst, c2d4 = _build_constants(
        np.asarray(centers, np.float32),
        np.asarray(sigmas, np.float32),
        np.asarray(coeffs, np.float32))

    if _NC_CACHE is None:
        _NC_CACHE = _build_bass()
    nc = _NC_CACHE

    in_maps = []
    for c in range(NCORES):
        xst, xrb = _pack(X[c * MC:(c + 1) * MC])
        in_maps.append({"xst": xst, "xrb": xrb, "cst": cst, "c2d4": c2d4})
    res = bass_utils.run_bass_kernel_spmd(nc, in_maps, list(range(NCORES)))
    return np.concatenate([r["yout"] for r in res.results], axis=0)


# revision 90
# speedup vs baseline: 1.6667x; 1.0045x over previous
"""ANFIS forward kernel for Trainium2, 8-core data-parallel. v8.

Algebra per row n (see reference):
    l_r = sum_d [2*c*a*x - a*x^2] - k_r;  s_r = exp(l_r)
    G_f = sum_r s_r * Chat[r,f]   (f = (o,i) products, i innermost)
    U_o = sum_i xhat_i * G_(o,i);  out = softmax_o(U / (S + eps))

Design (v8, 41.5us vs 68.7us v6 baseline under the TimelineSim model):
  - Host supplies the PE-transpose layout directly (xst f32r) and xhat
    in fp16 (xrb), killing all on-chip transposes.
  - Phase 1 computes every M1 batch (one 512-wide f32r matmul, 1 c/row)
    plus its ACT exp up front into sst_all, so the steady-state ACT
    queue is homogeneous copies (an exp interleaved between copies
    head-of-line-blocks the in-order ACT sequencer on its M1 dep).
  - M2 per group: two 512-col f32r matmuls (FPAD=256 bank layout) plus
    a 4-col S-matmul accumulating S into a per-meta PSUM tile seeded
    with eps by a ones128 @ (eps/128) matmul; one DVE reciprocal per
    meta replaces 32 per-group ACT extracts.
  - ACT copies G PSUM->SBUF with a cast to bf16 (680 cols, 752ns); the
    per-row multiply then runs on DVE in 2x_1p mode over group PAIRS
    (all-16-bit operands, i innermost packed; fp16 xhat is free
    accuracy); DIRECT pairs instead multiply fp32 straight from PSUM so
    the pipeline head does not wait for ACT.
  - i-reduction per pair: a DVE bf16 fold chain (the first three folds
    hit the 2x_1p mode, which tensor_reduce never can: 1049 vs 1477ns),
    or a Pool fp32 tensor_tensor fold chain for POOL_RED pairs (Pool
    cannot touch PSUM, but p4 is SBUF; Pool has no 2x so fp32 is free
    accuracy there). POOL_RED interleaved so neither engine backs up.
  - Softmax per meta (METAS sizes, small ones last to shorten the
    drain); each meta's tail is emitted one pair into the next meta so
    its e32 exp does not block the next meta's copies; per-meta stores.

Layout: row n of a core's 16384-row slice -> (p, t) = (n // 128, n % 128).
Group g = tiles [4g, 4g+4); metas cover METAS[i] groups each.
"""

import numpy as np

N, D, R, O = 131072, 16, 32, 10
EPS = 1e-8
NCORES = 8
MC = N // NCORES          # rows per core = 16384
TPG = 4                   # tiles (of 128 rows) per group
NG = 32                   # groups per core
MB = 4                    # groups per M1 batch
META = 8                  # groups per softmax batch
NMETA = NG // META        # 4

DI = D + 1                # 17: x dims + ones
F = O * DI                # 170 product features, f = o*17 + i
FPAD = 256                # per-j feature stride in g4 (bank alignment)
NT = MC // 128            # 128 tiles per core

# input chunks in groups: early small chunk -> early compute start
CH = [(0, 4), (4, 4), (8, 8), (16, 16)]

# softmax batches, in groups; smaller ones at the end shorten the drain
METAS = [4, 8, 8, 8, 4]

# group PAIRS whose i-reduction runs as a Pool fold chain (rest: DVE
# reduce). None in the last two metas: Pool drains slowly at the tail.
POOL_RED = frozenset({0, 4, 6, 8, 10, 12, 14})
# pairs that skip the ACT copy: DVE multiplies fp32 straight from PSUM.
# Used at the head where DVE is otherwise idle and every ACT slot counts.
DIRECT = frozenset({0, 4})
# pairs whose reduce is split: group A on Pool chain, group B on DVE
SPLIT_RED = frozenset()
# metas whose se-reduce runs on Pool (fold) instead of DVE
POOL_SE = frozenset()


def _build_constants(centers, sigmas, coeffs):
    a = 1.0 / (2.0 * sigmas.astype(np.float64) ** 2)          # (R,D)
    c = centers.astype(np.float64)

    # WL4: lhsT for M1. out partition (j,r) = j*32+r; input partition
    # (j,s,d) = j*32 + s*16 + d  (s=0: x, s=1: x^2).
    wl4 = np.zeros((128, 128), np.float64)
    for j in range(TPG):
        for r in range(R):
            pi = j * R + r
            for d in range(D):
                wl4[j * 32 + 0 * 16 + d, pi] = 2.0 * c[r, d] * a[r, d]   # x
                wl4[j * 32 + 1 * 16 + d, pi] = -a[r, d]                  # x^2
    negk = -(c * c * a).sum(axis=1)                            # (R,)
    negk4 = np.tile(negk, TPG).reshape(128, 1)

    # Chat (R, 170): f = o*17+i (i=16 -> bias row)
    chat = coeffs.astype(np.float64).transpose(0, 2, 1).reshape(R, F)
    # C2D4 (128, 1024): [(j,r), j'*256+f] = delta_jj' * chat[r,f]
    c2d4 = np.zeros((128, TPG * FPAD), np.float64)
    for j in range(TPG):
        c2d4[j * R:(j + 1) * R, j * FPAD:j * FPAD + F] = chat
    # sS rhs (128, 4): [(j,r), j'] = delta_jj' -> S_j per group
    srhs = np.zeros((128, TPG), np.float64)
    for j in range(TPG):
        srhs[j * R:(j + 1) * R, j] = 1.0
    cst = np.concatenate([negk4, wl4, srhs], axis=1)           # (128, 133)
    return cst.astype(np.float32), c2d4.astype(np.float32)


def _build_bass():
    import concourse.bacc as bacc
    import concourse.mybir as mybir
    from concourse.tile import TileContext

    f32 = mybir.dt.float32
    f32r = mybir.dt.float32r
    bf16 = mybir.dt.bfloat16
    AX = mybir.AxisListType
    ALU = mybir.AluOpType
    ACTF = mybir.ActivationFunctionType

    nc = bacc.Bacc("TRN2", target_bir_lowering=False, debug=False)
    xst_d = nc.declare_dram_parameter("xst", [128, NG * 128], f32r,
                                      isOutput=False)
    fp16 = mybir.dt.float16
    xrb_d = nc.declare_dram_parameter("xrb", [128, NT * DI], fp16,
                                      isOutput=False)
    cst_d = nc.declare_dram_parameter("cst", [128, 133], f32r, isOutput=False)
    c2d4_d = nc.declare_dram_parameter("c2d4", [128, TPG * FPAD], f32r,
                                       isOutput=False)
    yout = nc.declare_dram_parameter("yout", [MC, O], f32, isOutput=True)

    youtv = yout[:, :].rearrange("(p t) o -> p t o", p=128)

    with TileContext(nc) as tc:
        with (
            tc.tile_pool(name="const", bufs=1) as cpool,
            tc.tile_pool(name="work", bufs=6) as wpool,
            tc.tile_pool(name="meta", bufs=4) as mpool,
            tc.tile_pool(name="ps_l", bufs=1, space="PSUM") as ps_l,
            tc.tile_pool(name="ps_g", bufs=3, space="PSUM") as ps_g,
            tc.tile_pool(name="ps_s", bufs=1, space="PSUM") as ps_s,
        ):
            # constants: cst via the Pool SWDGE path (its descriptor-gen
            # doesn't hold HWDGE), xst chunk0 first in the HWDGE queue.
            cst = cpool.tile([128, 133], f32r)
            nc.gpsimd.dma_start(out=cst[:], in_=cst_d[:, :])
            negk4 = cst[:, 0:1]
            wl4 = cst[:, 1:129]
            srhs = cst[:, 129:133]
            xst = cpool.tile([128, NG * 128], f32r)
            xrb = cpool.tile([128, NT * DI], fp16)
            c2d4 = cpool.tile([128, TPG * FPAD], f32r)
            ones128 = cpool.tile([128, 128], f32)
            epscol = cpool.tile([128, R], f32)
            nc.vector.memset(ones128[:], 1.0)
            nc.vector.memset(epscol[:], EPS / 128.0)
            # hoist the ACT exp-table load out of the critical path
            dummy = cpool.tile([128, 1], f32)
            nc.scalar.activation(dummy[:], epscol[:, 0:1], ACTF.Exp)
            for i, (s, n) in enumerate(CH):
                nc.sync.dma_start(out=xst[:, s * 128:(s + n) * 128],
                                  in_=xst_d[:, s * 128:(s + n) * 128])
                nc.sync.dma_start(out=xrb[:, s * TPG * DI:(s + n) * TPG * DI],
                                  in_=xrb_d[:, s * TPG * DI:(s + n) * TPG * DI])
                if i == 0:
                    nc.sync.dma_start(out=c2d4[:, 0:512],
                                      in_=c2d4_d[:, 0:512])
                    nc.sync.dma_start(out=c2d4[:, 512:1024],
                                      in_=c2d4_d[:, 512:1024])
            xrv = xrb[:].rearrange("p (t c) -> p t c", c=DI)

            # ---- phase 1: all M1 batches + exps -> sst_all in SBUF -------
            # Keeps the steady-state ACT stream homogeneous (copies only):
            # an exp interleaved between copies stalls the whole in-order
            # ACT queue on its M1 dependency.
            sst_all = cpool.tile([128, NG * 128], f32r)
            gb = 0
            for nb in [4] * 8:
                w = nb * 128
                l16 = ps_l.tile([128, MB * 128], f32, tag="l16")
                for h in range(0, w, 512):
                    nc.tensor.matmul(
                        l16[:, h:min(h + 512, w)], lhsT=wl4,
                        rhs=xst[:, gb * 128 + h:gb * 128 + min(h + 512, w)],
                        start=True, stop=True)
                nc.scalar.activation(sst_all[:, gb * 128:gb * 128 + w],
                                     l16[:, 0:w], ACTF.Exp,
                                     bias=negk4, scale=1.0)
                gb += nb

            g0m = 0                 # first group of this meta
            pair_base = 0           # global pair index base
            pending_tail = None     # deferred softmax-tail emitter
            for m, SZ in enumerate(METAS):
                GJ = SZ * TPG       # tiles (and S-columns) in this meta
                sS = ps_s.tile([128, R], f32, tag="sS")
                # seed sS with eps: ones128^T @ epscol = eps everywhere
                nc.tensor.matmul(sS[:, 0:GJ], lhsT=ones128[:],
                                 rhs=epscol[:, 0:GJ],
                                 start=True, stop=False, skip_group_check=True)
                u32 = mpool.tile([128, META * TPG * O], f32, tag="u32")

                last_meta = (m == len(METAS) - 1)
                for P in range(SZ // 2):        # group pairs
                    if (P == min(1, SZ // 2 - 1) and not last_meta
                            and pending_tail is not None):
                        # emit the previous meta's softmax tail here so its
                        # e32 exp doesn't head-of-line-block this meta's
                        # copies in the in-order ACT queue
                        pending_tail()
                        pending_tail = None
                    JP = 2 * TPG                # 8 tiles per pair
                    direct = (P + pair_base) in DIRECT
                    if direct:
                        p4 = wpool.tile([128, JP * F], f32, tag="p4f")
                    else:
                        p4 = wpool.tile([128, JP * F], bf16, tag="p4")
                        gs = wpool.tile([128, JP * F], bf16, tag="gs")
                    for k in range(2):
                        q = P * 2 + k
                        g = g0m + q
                        sst_g = sst_all[:, g * 128:(g + 1) * 128]

                        g4 = ps_g.tile([128, TPG * FPAD], f32, tag="g4")
                        nc.tensor.matmul(g4[:, 0:512], lhsT=sst_g,
                                         rhs=c2d4[:, 0:512],
                                         start=True, stop=True)
                        nc.tensor.matmul(g4[:, 512:1024], lhsT=sst_g,
                                         rhs=c2d4[:, 512:1024],
                                         start=True, stop=True)
                        nc.tensor.matmul(sS[:, TPG * q:TPG * (q + 1)],
                                         lhsT=sst_g, rhs=srhs,
                                         start=False, stop=(q == SZ - 1),
                                         skip_group_check=True)
                        g4f = g4[:].rearrange("p (j f) -> p j f",
                                              j=TPG)[:, :, 0:F]
                        if direct:
                            # DVE fp32 multiply straight from PSUM
                            xh1 = xrv[:, TPG * g:TPG * (g + 1), :].unsqueeze(
                                2).broadcast_to([128, TPG, O, DI])
                            nc.vector.tensor_tensor(
                                p4[:, k * TPG * F:(k + 1) * TPG * F].rearrange(
                                    "p (j o i) -> p j o i", j=TPG, o=O),
                                g4f.rearrange("p j (o i) -> p j o i", o=O),
                                xh1, ALU.mult)
                        else:
                            # ACT: PSUM->SBUF gather of used cols, cast bf16
                            nc.scalar.activation(
                                gs[:, k * TPG * F:(k + 1) * TPG * F].rearrange(
                                    "p (j f) -> p j f", j=TPG),
                                g4f, ACTF.Copy)

                    g0 = g0m + P * 2
                    if not direct:
                        # DVE 2x multiply for the pair: p4 = gs * xhat
                        p4v = p4[:].rearrange("p (j o i) -> p j o i",
                                              j=JP, o=O)
                        gsv = gs[:].rearrange("p (j o i) -> p j o i",
                                              j=JP, o=O)
                        xhv = xrv[:, TPG * g0:TPG * (g0 + 2), :].unsqueeze(
                            2).broadcast_to([128, JP, O, DI])
                        nc.vector.tensor_tensor(p4v, gsv, xhv, ALU.mult)

                    # i-reduction -> u32[:, P*80:(P+1)*80]
                    uslice = u32[:, P * JP * O:(P + 1) * JP * O].rearrange(
                        "p (j o) -> p j o", j=JP)
                    p4jo = p4[:].rearrange("p (jo i) -> p jo i", i=DI)
                    if P + pair_base in SPLIT_RED:
                        half = TPG * O
                        nc.vector.tensor_reduce(
                            uslice[:, 0:TPG, :],
                            p4jo[:, 0:half, :], axis=AX.X, op=ALU.add)
                        pj = p4jo[:, half:2 * half, :]
                        us2 = u32[:, P * JP * O + half:(P + 1) * JP * O]
                        t8 = wpool.tile([128, JP * O * 8], f32, tag="t8")
                        t8v = t8[:, 0:half * 8].rearrange(
                            "p (jo i) -> p jo i", i=8)
                        nc.gpsimd.tensor_tensor(
                            t8v, pj[:, :, 0:8], pj[:, :, 8:16], ALU.add)
                        t4 = wpool.tile([128, JP * O * 4], f32, tag="t4")
                        t4v = t4[:, 0:half * 4].rearrange(
                            "p (jo i) -> p jo i", i=4)
                        nc.gpsimd.tensor_tensor(
                            t4v, t8v[:, :, 0:4], t8v[:, :, 4:8], ALU.add)
                        t2 = wpool.tile([128, JP * O * 2], f32, tag="t2")
                        t2v = t2[:, 0:half * 2].rearrange(
                            "p (jo i) -> p jo i", i=2)
                        nc.gpsimd.tensor_tensor(
                            t2v, t4v[:, :, 0:2], t4v[:, :, 2:4], ALU.add)
                        t1 = wpool.tile([128, JP * O], f32, tag="t1")
                        nc.gpsimd.tensor_tensor(
                            t1[:, 0:half],
                            t2v[:, :, 0:1].rearrange("p a b -> p (a b)"),
                            t2v[:, :, 1:2].rearrange("p a b -> p (a b)"),
                            ALU.add)
                        nc.gpsimd.tensor_tensor(
                            us2, t1[:, 0:half],
                            pj[:, :, 16:17].rearrange("p a b -> p (a b)"),
                            ALU.add)
                    elif P + pair_base in POOL_RED:
                        t8 = wpool.tile([128, JP * O * 8], f32, tag="t8")
                        t8v = t8[:].rearrange("p (jo i) -> p jo i", i=8)
                        nc.gpsimd.tensor_tensor(
                            t8v, p4jo[:, :, 0:8], p4jo[:, :, 8:16], ALU.add)
                        t4 = wpool.tile([128, JP * O * 4], f32, tag="t4")
                        t4v = t4[:].rearrange("p (jo i) -> p jo i", i=4)
                        nc.gpsimd.tensor_tensor(
                            t4v, t8v[:, :, 0:4], t8v[:, :, 4:8], ALU.add)
                        t2 = wpool.tile([128, JP * O * 2], f32, tag="t2")
                        t2v = t2[:].rearrange("p (jo i) -> p jo i", i=2)
                        nc.gpsimd.tensor_tensor(
                            t2v, t4v[:, :, 0:2], t4v[:, :, 2:4], ALU.add)
                        t1 = wpool.tile([128, JP * O], f32, tag="t1")
                        nc.gpsimd.tensor_tensor(
                            t1[:],
                            t2v[:, :, 0:1].rearrange("p a b -> p (a b)"),
                            t2v[:, :, 1:2].rearrange("p a b -> p (a b)"),
                            ALU.add)
                        nc.gpsimd.tensor_tensor(
                            uslice.rearrange("p a b -> p (a b)"), t1[:],
                            p4jo[:, :, 16:17].rearrange("p a b -> p (a b)"),
                            ALU.add)
                    else:
                        # DVE bf16 fold chain: the first three folds run in
                        # the 2x_1p mode (all-bf16 packed), which
                        # tensor_reduce can never use -> 1049ns vs 1477ns
                        d8 = wpool.tile([128, JP * O * 8], bf16, tag="d8")
                        d8v = d8[:].rearrange("p (jo i) -> p jo i", i=8)
                        nc.vector.tensor_tensor(
                            d8v, p4jo[:, :, 0:8], p4jo[:, :, 8:16], ALU.add)
                        d4 = wpool.tile([128, JP * O * 4], bf16, tag="d4")
                        d4v = d4[:].rearrange("p (jo i) -> p jo i", i=4)
                        nc.vector.tensor_tensor(
                            d4v, d8v[:, :, 0:4], d8v[:, :, 4:8], ALU.add)
                        d2 = wpool.tile([128, JP * O * 2], bf16, tag="d2")
                        d2v = d2[:].rearrange("p (jo i) -> p jo i", i=2)
                        nc.vector.tensor_tensor(
                            d2v, d4v[:, :, 0:2], d4v[:, :, 2:4], ALU.add)
                        d1 = wpool.tile([128, JP * O], f32, tag="d1")
                        nc.vector.tensor_tensor(
                            d1[:],
                            d2v[:, :, 0:1].rearrange("p a b -> p (a b)"),
                            d2v[:, :, 1:2].rearrange("p a b -> p (a b)"),
                            ALU.add)
                        nc.vector.tensor_tensor(
                            uslice.rearrange("p a b -> p (a b)"), d1[:],
                            p4jo[:, :, 16:17].rearrange("p a b -> p (a b)"),
                            ALU.add)

                def make_tail(m=m, SZ=SZ, GJ=GJ, g0m=g0m, sS=sS, u32=u32):
                    def tail():
                        s32r = mpool.tile([128, R], f32, tag="s32r")
                        nc.vector.reciprocal(s32r[:, 0:GJ], sS[:, 0:GJ])
                        uv = u32[:, 0:GJ * O]
                        u32v = uv.rearrange("p (g o) -> p g o", o=O)
                        s32b = s32r[:, 0:GJ].unsqueeze(2).broadcast_to(
                            [128, GJ, O])
                        eng_un = nc.gpsimd if m in POOL_SE else nc.vector
                        eng_un.tensor_tensor(u32v, u32v, s32b, ALU.mult)
                        e32 = mpool.tile([128, META * TPG * O], f32,
                                         tag="e32")
                        ev = e32[:, 0:GJ * O].rearrange("p (g o) -> p g o",
                                                        o=O)
                        nc.scalar.activation(e32[:, 0:GJ * O], uv, ACTF.Exp)
                        se32 = mpool.tile([128, R], f32, tag="se32")
                        nc.vector.tensor_reduce(se32[:, 0:GJ], ev,
                                                axis=AX.X, op=ALU.add)
                        nc.vector.reciprocal(se32[:, 0:GJ], se32[:, 0:GJ])
                        se32b = se32[:, 0:GJ].unsqueeze(2).broadcast_to(
                            [128, GJ, O])
                        o_all = mpool.tile([128, META * TPG * O], f32,
                                           tag="oall")
                        nc.vector.tensor_tensor(
                            o_all[:, 0:GJ * O].rearrange("p (g o) -> p g o",
                                                         o=O),
                            ev, se32b, ALU.mult)
                        nc.sync.dma_start(
                            out=youtv[:, TPG * g0m:TPG * (g0m + SZ), :],
                            in_=o_all[:, 0:GJ * O].rearrange(
                                "p (t o) -> p t o", o=O))
                    return tail

                if pending_tail is not None:
                    # last meta: flush the previous tail only after ALL its
                    # copies are emitted, so they aren't blocked behind it
                    pending_tail()
                pending_tail = make_tail()
                g0m += SZ
                pair_base += SZ // 2
            pending_tail()
    nc.compile()
    return nc


def _pack(Xc):
    """Per-core host packing.

    xst (128, 4096) f32: partition (j,s,d) = j*32+s*16+d, col g*128+p,
        value x[n,d]^(s+1) for row n = p*128 + (4g+j).
    xrb (128, 2176) fp16: [p, t*17+i] = xhat for row n = p*128+t.
    """
    X3 = Xc.reshape(128, NT, D)                       # [p, t, d]
    A = X3.transpose(1, 2, 0)                         # [t, d, p]
    G4v = A.reshape(NG, TPG, D, 128)                  # [g, j, d, p]
    B = np.stack([G4v, G4v * G4v], axis=2)            # [g, j, s, d, p]
    xst = np.ascontiguousarray(
        B.transpose(1, 2, 3, 0, 4).reshape(128, NG * 128))
    xr = np.concatenate([X3, np.ones((128, NT, 1), np.float32)], axis=2)
    xrb = np.ascontiguousarray(xr.reshape(128, NT * DI)).astype(np.float16)
    return xst, xrb


_NC_CACHE = None


def kernel(X, centers, sigmas, coeffs):
    global _NC_CACHE
    from concourse import bass_utils

    X = np.asarray(X, np.float32)
    cst, c2d4 = _build_constants(
        np.asarray(centers, np.float32),
        np.asarray(sigmas, np.float32),
        np.asarray(coeffs, np.float32))

    if _NC_CACHE is None:
        _NC_CACHE = _build_bass()
    nc = _NC_CACHE

    in_maps = []
    for c in range(NCORES):
        xst, xrb = _pack(X[c * MC:(c + 1) * MC])
        in_maps.append({"xst": xst, "xrb": xrb, "cst": cst, "c2d4": c2d4})
    res = bass_utils.run_bass_kernel_spmd(nc, in_maps, list(range(NCORES)))
    return np.concatenate([r["yout"] for r in res.results], axis=0)
